# revision 1
# baseline (speedup 1.0000x reference)
import sys

sys.path.insert(0, "/opt/trn_rl_repo")

import numpy as np
import ml_dtypes

import concourse.bass as bass
import concourse.mybir as mybir
import concourse.tile as tile
from concourse import bacc
from concourse import bass_utils
from concourse.masks import make_identity

B, S, E, H = 256, 128, 512, 512
NC = 8
BS = B // NC          # batch per core = 32
H4 = 4 * H            # 2048
HT = H // 128         # 4 h-tiles
TC = 32               # decoder t-chunk for attention
NTC = S // TC         # 4
C_EXP = 10.0

F32 = mybir.dt.float32
BF16 = mybir.dt.bfloat16
AF = mybir.ActivationFunctionType
OP = mybir.AluOpType

BF = ml_dtypes.bfloat16


def _prep(inputs, target, embedding, enc_Wih, enc_Whh, enc_b,
          dec_Wih, dec_Whh, dec_b,
          g_Wq, g_bq, g_Wref, g_bref, g_V,
          p_Wq, p_bq, p_Wref, p_bref, p_V, dec_start):
    """Host-side weight preprocessing. Gate order reordered i,f,g,o -> i,f,o,g.
    State convention: kernel carries Hs=2h, Cs=2c; 0.5 factors folded into
    weights that consume h (Whh, g_Wq, g_Wref, p_Wref)."""
    perm = np.concatenate([np.arange(0, H), np.arange(H, 2 * H),
                           np.arange(3 * H, 4 * H), np.arange(2 * H, 3 * H)])
    out = {}
    # recurrent weights as matmul rhs [K=512, N=2048], halved, gate-permuted
    out["Wenc"] = np.ascontiguousarray((0.5 * enc_Whh[perm, :]).T).astype(BF)
    out["Wdec"] = np.ascontiguousarray((0.5 * dec_Whh[perm, :]).T).astype(BF)
    # per-vocab input projections (+bias), gate-permuted  [128, 2048]
    out["Penc"] = ((embedding @ enc_Wih.T + enc_b)[:, perm]).astype(BF)
    out["Pdec"] = ((embedding @ dec_Wih.T + dec_b)[:, perm]).astype(BF)
    out["xd0"] = ((dec_Wih @ dec_start + dec_b)[perm])[None, :].astype(BF)
    # attention weights as lhsT [K_in=512, M_out=512]
    out["Wqg"] = np.ascontiguousarray((0.5 * g_Wq).T).astype(BF)
    out["Wqp"] = np.ascontiguousarray(p_Wq.T).astype(BF)
    out["Wrg"] = np.ascontiguousarray((0.5 * g_Wref).T).astype(BF)
    out["Wrp"] = np.ascontiguousarray((0.5 * p_Wref).T).astype(BF)
    # biases packed [128, HT] (column m = m-th 128-slice)
    for nm, v in (("bqg", g_bq), ("bqp", p_bq), ("brg", g_bref), ("brp", p_bref)):
        out[nm] = np.ascontiguousarray(v.reshape(HT, 128).T).astype(np.float32)
    for nm, v in (("Vg", g_V), ("Vp", p_V)):
        out[nm] = np.ascontiguousarray(v.reshape(HT, 128).T).astype(BF)
    out["ones1"] = np.ones((1, BS), dtype=BF)
    out["ones32"] = np.ones((BS, 1), dtype=np.float32)
    return out


def _build(nc, t_in):
    """Emit the tile program. t_in: dict name -> dram tensor handle."""
    loss_out = nc.dram_tensor("loss_out", [1, 1], F32, kind="ExternalOutput")
    refp_dram = nc.dram_tensor("refp_stage", [128, HT, BS, S], BF16,
                               kind="Internal")

    with tile.TileContext(nc) as tc2:
        ctx = tc2
        with (
            tc2.tile_pool(name="weights", bufs=1) as wp,
            tc2.tile_pool(name="bigbuf", bufs=1) as bigp,
            tc2.tile_pool(name="state", bufs=2) as stp,
            tc2.tile_pool(name="smalls", bufs=2) as smp,
        ):
            # ---- load constants ----
            def load(name, shape, dt):
                t = wp.tile(shape, dt, tag=name)
                nc.sync.dma_start(t[:], t_in[name].ap())
                return t

            Wenc = load("Wenc", [128, HT, H4], BF16)
            Wdec = load("Wdec", [128, HT, H4], BF16)
            Penc = load("Penc", [128, H4], BF16)
            Pdec = load("Pdec", [128, H4], BF16)
            xd0 = load("xd0", [1, H4], BF16)
            Wqg = load("Wqg", [128, HT, H], BF16)
            Wqp = load("Wqp", [128, HT, H], BF16)
            Wrg = load("Wrg", [128, HT, H], BF16)
            Wrp = load("Wrp", [128, HT, H], BF16)
            bqg = load("bqg", [128, HT], F32)
            bqp = load("bqp", [128, HT], F32)
            brg = load("brg", [128, HT], F32)
            brp = load("brp", [128, HT], F32)
            Vg = load("Vg", [128, HT], BF16)
            Vp = load("Vp", [128, HT], BF16)
            ones1 = load("ones1", [1, BS], BF16)
            ones32 = load("ones32", [BS, 1], F32)
            OHdec = load("oh_dec", [128, BS], BF16)

            idn = wp.tile([128, 128], BF16)
            make_identity(nc, idn[:])

            # ---- big persistent buffers ----
            refg = bigp.tile([128, HT, BS, S], BF16)     # 4MB
            Hdec = bigp.tile([128, HT, BS, S], BF16)     # 4MB
            S_all = bigp.tile([BS, S], F32)
            T_all = bigp.tile([BS, S], F32)

            # ---- LSTM chain ----
            def lstm_chain(Wrec, n_steps, is_enc, Hst0, Cst0):
                Hst, Cst = Hst0, Cst0
                with (
                    tc2.tile_pool(name="gpsum", bufs=1,
                                  space=bass.MemorySpace.PSUM) as gp,
                    tc2.tile_pool(name="trpsum", bufs=2,
                                  space=bass.MemorySpace.PSUM) as trp,
                    tc2.tile_pool(name="refpsum", bufs=2,
                                  space=bass.MemorySpace.PSUM) as rfp,
                    tc2.tile_pool(name="hrec", bufs=2) as hrp,
                    tc2.tile_pool(name="cell", bufs=2) as cp,
                    tc2.tile_pool(name="ohp", bufs=4) as ohp,
                ):
                    Hrec = None
                    for t in range(n_steps):
                        if is_enc and t % 16 == 0:
                            Hrec = hrp.tile([128, HT, BS, 16], BF16, tag="hrec")
                        gates = gp.tile([BS, H4], F32)
                        if is_enc:
                            ohe = ohp.tile([128, BS], BF16, tag="ohe")
                            nc.sync.dma_start(ohe[:],
                                              t_in["oh_enc"].ap()[:, t, :])
                        # x-term matmul first (starts accumulation)
                        for n in range(4):
                            nsl = bass.ts(n, 512)
                            if is_enc:
                                nc.tensor.matmul(gates[:, nsl],
                                                 ohe[:], Penc[:, nsl],
                                                 start=True, stop=False)
                            elif t == 0:
                                nc.tensor.matmul(gates[:, nsl],
                                                 ones1[:], xd0[:, nsl],
                                                 start=True, stop=False)
                            else:
                                nc.tensor.matmul(gates[:, nsl],
                                                 OHdec[:], Pdec[:, nsl],
                                                 start=True, stop=False)
                        for k in range(HT):
                            for n in range(4):
                                nsl = bass.ts(n, 512)
                                nc.tensor.matmul(gates[:, nsl],
                                                 Hst[:, k, :],
                                                 Wrec[:, k, nsl],
                                                 start=False, stop=(k == HT - 1))
                        # nonlinearities: sigma(x) = 0.5*(1+tanh(x/2)) folding
                        tifo = cp.tile([BS, 3 * H], BF16, tag="tifo")
                        nc.scalar.activation(tifo[:], gates[:, 0:3 * H],
                                             AF.Tanh, scale=0.5)
                        tg = cp.tile([BS, H], BF16, tag="tg")
                        nc.scalar.activation(tg[:], gates[:, 3 * H:4 * H], AF.Tanh)
                        ti = tifo[:, 0:H]
                        tf = tifo[:, H:2 * H]
                        to = tifo[:, 2 * H:3 * H]
                        # C' = 0.5*(1+tf)*C + (1+ti)*tg
                        A = cp.tile([BS, H], F32, tag="A")
                        nc.vector.scalar_tensor_tensor(A[:], tf, 1.0, Cst[:],
                                                       op0=OP.add, op1=OP.mult)
                        Bt = cp.tile([BS, H], F32, tag="B")
                        nc.vector.scalar_tensor_tensor(Bt[:], ti, 1.0, tg[:],
                                                       op0=OP.add, op1=OP.mult)
                        Cn = stp.tile([BS, H], F32, tag="C")
                        nc.vector.scalar_tensor_tensor(Cn[:], A[:], 0.5, Bt[:],
                                                       op0=OP.mult, op1=OP.add)
                        th = cp.tile([BS, H], BF16, tag="th")
                        nc.scalar.activation(th[:], Cn[:], AF.Tanh, scale=0.5)
                        Hb = cp.tile([BS, H], BF16, tag="Hb")
                        nc.vector.scalar_tensor_tensor(Hb[:], to, 1.0, th[:],
                                                       op0=OP.add, op1=OP.mult)
                        # transpose H' back to [h, b]
                        trt = trp.tile([128, HT, BS], BF16)
                        for k in range(HT):
                            nc.tensor.transpose(trt[:, k, :],
                                                Hb[:, bass.ts(k, 128)],
                                                idn[0:BS, 0:BS])
                        Hn = stp.tile([128, HT, BS], BF16, tag="H")
                        nc.scalar.copy(Hn[:], trt[:])
                        if is_enc:
                            nc.vector.tensor_copy(Hrec[:, :, :, t % 16], trt[:])
                        else:
                            nc.vector.tensor_copy(Hdec[:, :, :, t], trt[:])
                        Hst, Cst = Hn, Cn
                        # every 16 encoder steps: project refs for those columns
                        if is_enc and t % 16 == 15:
                            g0 = t - 15
                            for which in range(2):
                                Wr = Wrg if which == 0 else Wrp
                                br = brg if which == 0 else brp
                                for m in range(HT):
                                    pr = rfp.tile([128, BS * 16], F32)
                                    for k in range(HT):
                                        nc.tensor.matmul(
                                            pr[:],
                                            Wr[:, k, bass.ts(m, 128)],
                                            Hrec[:, k, :, :].rearrange(
                                                "p b t -> p (b t)"),
                                            start=(k == 0), stop=(k == HT - 1))
                                    prv = pr[:].rearrange("p (b t) -> p b t", b=BS)
                                    if which == 0:
                                        nc.scalar.activation(
                                            refg[:, m, :, g0:g0 + 16], prv,
                                            AF.Identity, bias=br[:, m:m + 1])
                                    else:
                                        stg = smp.tile([128, BS, 16], BF16,
                                                       tag="refstg")
                                        nc.scalar.activation(
                                            stg[:], prv,
                                            AF.Identity, bias=br[:, m:m + 1])
                                        nc.sync.dma_start(
                                            refp_dram.ap()[:, m, :, g0:g0 + 16],
                                            stg[:])
                return Hst, Cst

            Hz = stp.tile([128, HT, BS], BF16, tag="H")
            nc.gpsimd.memset(Hz[:], 0.0)
            Cz = stp.tile([BS, H], F32, tag="C")
            nc.gpsimd.memset(Cz[:], 0.0)
            Hst, Cst = lstm_chain(Wenc, S, True, Hz, Cz)
            _, _ = lstm_chain(Wdec, S, False, Hst, Cst)

            # ---- attention ----
            with (
                tc2.tile_pool(name="lpsum", bufs=1,
                              space=bass.MemorySpace.PSUM) as lp_pool,
                tc2.tile_pool(name="qppsum", bufs=1,
                              space=bass.MemorySpace.PSUM) as qpp,
                tc2.tile_pool(name="smpsum", bufs=1,
                              space=bass.MemorySpace.PSUM) as smps,
                tc2.tile_pool(name="xbuf", bufs=2) as xbp,
                tc2.tile_pool(name="qpbuf", bufs=1) as qpb,
                tc2.tile_pool(name="attn", bufs=2) as atp,
                tc2.tile_pool(name="refpb", bufs=2) as rpb,
            ):
                refp_b_next = rpb.tile([128, HT, S], BF16, tag="refpb")
                nc.sync.dma_start(refp_b_next[:], refp_dram.ap()[:, :, 0, :])

                def batched_qp(Wl, bias, rhs_view, out_bf):
                    # rhs_view: [128, HT, BS, TC]; out_bf: [128, HT, BS, TC]
                    nb = 512 // TC
                    for m in range(HT):
                        for n2 in range(BS // nb):
                            bsl = bass.ts(n2, nb)
                            ps = qpp.tile([128, nb, TC], F32, tag="qp_ps")
                            for k in range(HT):
                                nc.tensor.matmul(ps[:],
                                                 Wl[:, k, bass.ts(m, 128)],
                                                 rhs_view[:, k, bsl, :],
                                                 start=(k == 0),
                                                 stop=(k == HT - 1))
                            nc.scalar.activation(
                                out_bf[:, m, bsl, :],
                                ps[:], AF.Identity, bias=bias[:, m:m + 1])

                def attn_unit(qp_sb, ref_sb, Vw, b, lg_sb):
                    # additive attention logits for batch b, all TC t's
                    for hf in range(2):
                        lflat = atp.tile([1, 16 * 128], F32, tag="lflat")
                        lps = lp_pool.tile([1, 16 * 128], F32, tag="lps")
                        for m in range(HT):
                            xg = xbp.tile([128, 16, 128], BF16, tag="xadd")
                            qv = qp_sb[:, m, b, hf * 16:(hf + 1) * 16]
                            qv = qv.unsqueeze(2).broadcast_to([128, 16, 128])
                            rv = ref_sb[:, m, :].unsqueeze(1).broadcast_to(
                                [128, 16, 128])
                            nc.vector.tensor_tensor(xg[:], qv, rv, op=OP.add)
                            nc.scalar.activation(xg[:], xg[:], AF.Tanh)
                            xtv = xg[:].rearrange("p t s -> p (t s)")
                            for c in range(4):
                                nc.tensor.matmul(lps[:, bass.ts(c, 512)],
                                                 Vw[:, m:m + 1],
                                                 xtv[:, bass.ts(c, 512)],
                                                 start=(m == 0),
                                                 stop=(m == HT - 1))
                        nc.scalar.copy(lflat[:], lps[:])
                        nc.sync.dma_start(lg_sb[hf * 16:(hf + 1) * 16, :],
                                          lflat[:])

                for tcn in range(NTC):
                    tsl = bass.ts(tcn, TC)
                    QPg = qpb.tile([128, HT, BS, TC], BF16, tag="QPg")
                    batched_qp(Wqg, bqg, Hdec[:, :, :, tsl], QPg)
                    qall = qpb.tile([128, HT, BS, TC], BF16, tag="qall")
                    for b in range(BS):
                        # transpose ref_g[b] -> [s, h] as weighted-sum lhsT
                        rgt = atp.tile([128, HT, 128], BF16, tag="rgt")
                        rps = smps.tile([128, HT, 128], BF16, tag="rps")
                        for m in range(HT):
                            nc.tensor.transpose(rps[:, m, :], refg[:, m, b, :],
                                                idn[:])
                        nc.scalar.copy(rgt[:], rps[:])
                        lg = atp.tile([TC, 128], F32, tag="lg")
                        attn_unit(QPg, refg[:, :, b, :], Vg, b, lg)
                        e = atp.tile([TC, 128], BF16, tag="e")
                        Sg = atp.tile([TC, 1], F32, tag="Sg")
                        nc.scalar.activation(e[:], lg[:], AF.Exp,
                                             accum_out=Sg[:])
                        rS = atp.tile([TC, 1], F32, tag="rS")
                        nc.vector.reciprocal(rS[:], Sg[:])
                        a = atp.tile([TC, 128], BF16, tag="a")
                        nc.vector.tensor_scalar(a[:], e[:], rS[:], None,
                                                op0=OP.mult)
                        aps = smps.tile([128, TC], BF16, tag="aps")
                        nc.tensor.transpose(aps[:], a[:], idn[0:TC, 0:TC])
                        asb = atp.tile([128, TC], BF16, tag="asb")
                        nc.scalar.copy(asb[:], aps[:])
                        qps = smps.tile([128, HT, TC], F32, tag="qps")
                        for m in range(HT):
                            nc.tensor.matmul(qps[:, m, :],
                                             rgt[:, m, :], asb[:],
                                             start=True, stop=True)
                        nc.scalar.copy(qall[:, :, b, :], qps[:])
                    QPp = qpb.tile([128, HT, BS, TC], BF16, tag="QPp")
                    batched_qp(Wqp, bqp, qall[:], QPp)
                    for b in range(BS):
                        refp_b = refp_b_next
                        nb = tcn * BS + b + 1
                        if nb < NTC * BS:
                            refp_b_next = rpb.tile([128, HT, S], BF16,
                                                   tag="refpb")
                            nc.sync.dma_start(
                                refp_b_next[:],
                                refp_dram.ap()[:, :, nb % BS, :])
                        lg = atp.tile([TC, 128], F32, tag="lg")
                        attn_unit(QPp, refp_b, Vp, b, lg)
                        ltan = atp.tile([TC, 128], F32, tag="ltan")
                        nc.scalar.activation(ltan[:], lg[:], AF.Tanh)
                        col = tcn * BS + b
                        edump = atp.tile([TC, 128], BF16, tag="edump")
                        nc.scalar.activation(edump[:], ltan[:], AF.Exp,
                                             scale=C_EXP,
                                             accum_out=S_all[:, col:col + 1])
                        ohtb = atp.tile([TC, 128], F32, tag="ohtb")
                        nc.sync.dma_start(
                            ohtb[:],
                            t_in["oh_tgt"].ap()[b:b + 1, :].broadcast_to(
                                [TC, 128]))
                        tt = atp.tile([TC, 128], F32, tag="tt")
                        nc.vector.tensor_tensor(tt[:], ltan[:], ohtb[:],
                                                op=OP.mult)
                        nc.vector.tensor_reduce(T_all[:, col:col + 1], tt[:],
                                                axis=mybir.AxisListType.X,
                                                op=OP.add)

            # ---- loss tail (own pool scope) ----
            with (
                tc2.tile_pool(name="ltail", bufs=1) as ltp,
                tc2.tile_pool(name="ltpsum", bufs=1,
                              space=bass.MemorySpace.PSUM) as ltps,
            ):
                lse = ltp.tile([BS, S], F32, tag="lse")
                nc.scalar.activation(lse[:], S_all[:], AF.Ln)
                D = ltp.tile([BS, S], F32, tag="D")
                nc.vector.scalar_tensor_tensor(D[:], T_all[:], -C_EXP, lse[:],
                                               op0=OP.mult, op1=OP.add)
                red = ltp.tile([BS, 1], F32, tag="red")
                nc.vector.tensor_reduce(red[:], D[:],
                                        axis=mybir.AxisListType.X, op=OP.add)
                tot = ltps.tile([1, 1], F32, tag="tot")
                nc.tensor.matmul(tot[:], ones32[:], red[:],
                                 start=True, stop=True)
                tsb = ltp.tile([1, 1], F32, tag="tsb")
                nc.vector.tensor_copy(tsb[:], tot[:])
                nc.sync.dma_start(loss_out.ap(), tsb[:])
    return loss_out


def kernel(**inputs):
    np_in = {k: np.asarray(v) for k, v in inputs.items()}
    prep = _prep(**np_in)
    inp = np_in["inputs"].astype(np.int64)
    tgt = np_in["target"].astype(np.int64)

    nc = bacc.Bacc("TRN2", target_bir_lowering=False, debug=False,
                   num_devices=NC)
    t_in = {}
    shapes = {
        "Wenc": ([128, HT, H4], BF16), "Wdec": ([128, HT, H4], BF16),
        "Penc": ([128, H4], BF16), "Pdec": ([128, H4], BF16),
        "xd0": ([1, H4], BF16),
        "Wqg": ([128, HT, H], BF16), "Wqp": ([128, HT, H], BF16),
        "Wrg": ([128, HT, H], BF16), "Wrp": ([128, HT, H], BF16),
        "bqg": ([128, HT], F32), "bqp": ([128, HT], F32),
        "brg": ([128, HT], F32), "brp": ([128, HT], F32),
        "Vg": ([128, HT], BF16), "Vp": ([128, HT], BF16),
        "ones1": ([1, BS], BF16), "ones32": ([BS, 1], F32),
        "oh_enc": ([128, S, BS], BF16), "oh_dec": ([128, BS], BF16),
        "oh_tgt": ([BS, S], F32),
    }
    for nm, (shp, dt) in shapes.items():
        t_in[nm] = nc.dram_tensor(nm, shp, dt, kind="ExternalInput")

    _build(nc, t_in)
    nc.compile()

    vocab = np.arange(128)
    in_maps = []
    for c in range(NC):
        bsl = slice(c * BS, (c + 1) * BS)
        m = {}
        for nm in shapes:
            if nm in prep:
                m[nm] = np.ascontiguousarray(prep[nm])
        # one-hots: oh_enc[v, t, b] = (inputs[b, t] == v)
        ohe = (inp[bsl, :].T[None, :, :] == vocab[:, None, None])
        m["oh_enc"] = np.ascontiguousarray(ohe).astype(BF)
        ohd = (tgt[bsl, 0][None, :] == vocab[:, None])
        m["oh_dec"] = np.ascontiguousarray(ohd).astype(BF)
        oht = (tgt[bsl, 0][:, None] == vocab[None, :])
        m["oh_tgt"] = np.ascontiguousarray(oht).astype(np.float32)
        in_maps.append(m)

    res = bass_utils.run_bass_kernel_spmd(nc, in_maps,
                                          core_ids=list(range(NC)))
    total = sum(float(res.results[c]["loss_out"][0, 0]) for c in range(NC))
    return np.float32(total / (B * S))

